# revision 3
# baseline (speedup 1.0000x reference)
"""Trainium2 Bass kernel for nn_PopcntLayer (segment_reduce).

Computation: out[b,o] = sigmoid( sum_p x[b, sel[o,p]] * sigmoid(w[o,p]) - bias[o] )
 with x [1024, 4096] f32, sel [4096, 64] i32, w [4096, 64] f32, bias [4096] f32.

Strategy (output-width sharded across 8 cores, 512 outputs each):
  out = sigmoid(x @ A - bias) where A[i, o] = sum_{p: sel[o,p]=i} sigmoid(w[o,p])
  is a sparse (64 nnz per column) matrix applied as a dense [4096, 512] f16
  operand on the PE.

Host prep is index/layout-only: raw weights are scattered into a dense
[4096, 512] f16 "image" (empty cells = -30000, so sigmoid maps them to 0),
and input rows are permuted per-core so rows containing duplicate (i, o)
cells land in the last D_DIRTY chunks; duplicates' 2nd/3rd weights go to
small overflow images (img2/img3). On device, ACT applies sigmoid to each
128-row image chunk to produce the A-chunk (overflow layers are sigmoided
and added with DVE), and the PE accumulates out.T[o,b] += A_k.T @ xT_k.

The batch is processed in staggered phases [640, 256, 128] so the psum ->
sigmoid -> DMA epilogues of early phases overlap the matmul stream of later
ones, shrinking the kernel tail. DMA issue is spread across the SP (images,
outputs) and DVE (xT) queues; junk matmuls keep the PE p-state ramp warm
until the first A-chunk is ready.

The kernel computes out.T per core ([512, 1024] f16); host concatenates and
transposes back.
"""

import os
import sys

for _p in ("/opt/trn_rl_repo", "/root/.axon_site/_ro/trn_rl_repo"):
    if os.path.isdir(_p) and _p not in sys.path:
        sys.path.append(_p)

import numpy as np

import concourse.bass as bass
import concourse.tile as tile
import concourse.mybir as mybir
from concourse import bacc
from concourse import bass_utils

B = 1024          # batch
I = 4096          # input width
O = 4096          # output width
POP = 64          # popcount width
NCORES = 8
OSH = O // NCORES     # 512 outputs per core
KCH = I // 128        # 32 contraction chunks
OC = OSH // 128       # 4 output chunks per core
D_DIRTY = 3           # trailing chunks holding rows with duplicate cells
D3 = 1                # trailing chunks holding multiplicity-3 rows
NEG = -30000.0        # sigmoid(NEG) == 0 in f32/f16

# batch phases: staggered so epilogues overlap later phases' matmuls
QPH = [(0, 640), (640, 256), (896, 128)]
# junk warmup matmuls: keep PE busy (p-state ramp) until first A chunk lands
N_JUNK = 7
JUNK_F = 512

_CACHE = {}


def _build():
    """Build + compile the (SPMD, identical on all cores) Bass program."""
    if "nc" in _CACHE:
        return _CACHE["nc"]
    f32 = mybir.dt.float32
    f16 = mybir.dt.float16
    AF = mybir.ActivationFunctionType

    nc = bacc.Bacc("TRN2", debug=False)
    xT_d = nc.dram_tensor("xT", [I, B], f16, kind="ExternalInput")
    im1_d = nc.dram_tensor("img1", [I, OSH], f16, kind="ExternalInput")
    im2_d = nc.dram_tensor("img2", [D_DIRTY * 128, OSH], f16, kind="ExternalInput")
    im3_d = nc.dram_tensor("img3", [D3 * 128, OSH], f16, kind="ExternalInput")
    bia_d = nc.dram_tensor("bias", [128, OC], f32, kind="ExternalInput")
    out_d = nc.dram_tensor("outT", [OSH, B], f16, kind="ExternalOutput")

    # chunk batches for DMA: first batch is 1 chunk (fast head), rest 2
    batches = [[0]]
    k = 1
    while k < KCH:
        batches.append(list(range(k, min(k + 2, KCH))))
        k += 2

    with tile.TileContext(nc) as tc:
        with (
            tc.tile_pool(name="const", bufs=1) as constp,
            tc.tile_pool(name="xt", bufs=len(batches)) as xtp,
            tc.tile_pool(name="img", bufs=4) as imgp,
            tc.tile_pool(name="ak", bufs=KCH) as akp,
            tc.tile_pool(name="tmp", bufs=2) as tmpp,
            tc.tile_pool(name="ps", bufs=1, space="PSUM") as psp,
            tc.tile_pool(name="ob", bufs=4) as obp,
        ):
            pss = [
                psp.tile([128, B], f32, tag=f"ps{oc}", name=f"ps{oc}")
                for oc in range(OC)
            ]

            # Warm tile + junk matmuls: PE busy from ~0.5us so the p-state
            # ramp completes by the time real matmuls start.
            warm = constp.tile([128, JUNK_F], f16)
            nc.vector.memset(warm[:], 0.0)
            for _ in range(N_JUNK):
                nc.tensor.matmul(
                    pss[0][:, 0:JUNK_F], warm[:, 0:128], warm[:], start=True,
                    stop=True,
                )

            # img2/img3 overflow layers + bias: issued on DVE queue after the
            # first xT chunk (needed only near the end of the chunk stream).
            im2_sb = constp.tile([128, D_DIRTY, OSH], f16, tag="im2", name="im2")
            im3_sb = constp.tile([128, D3, OSH], f16, tag="im3", name="im3")
            bia_sb = constp.tile([128, OC], f32)

            xts = [None] * KCH      # (tile, idx) per chunk
            aks = [None] * KCH
            first_dirty = KCH - D_DIRTY

            for bi, bk in enumerate(batches):
                n = len(bk)
                k0 = bk[0]
                # image chunk batch via SP queue
                img_t = imgp.tile([128, n, OSH], f16, tag="img")
                nc.sync.dma_start(
                    img_t[:],
                    im1_d.ap()[k0 * 128 : (k0 + n) * 128, :].rearrange(
                        "(c p) o -> p c o", p=128
                    ),
                )
                # xT chunk batch via DVE queue
                xt_t = xtp.tile([128, n, B], f16, tag="xt", name=f"xt{bi}")
                nc.gpsimd.dma_start(
                    xt_t[:],
                    xT_d.ap()[k0 * 128 : (k0 + n) * 128, :].rearrange(
                        "(c p) b -> p c b", p=128
                    ),
                )
                if bi == 0:
                    # overflow layers + bias, behind the first xT chunk
                    nc.gpsimd.dma_start(
                        im2_sb[:],
                        im2_d.ap().rearrange("(c p) o -> p c o", p=128),
                    )
                    nc.gpsimd.dma_start(
                        im3_sb[:],
                        im3_d.ap().rearrange("(c p) o -> p c o", p=128),
                    )
                    nc.gpsimd.dma_start(bia_sb[:], bia_d.ap())
                for c, k in enumerate(bk):
                    xts[k] = (xt_t, c)
                    ak = akp.tile([128, OSH], f16, tag="ak", name=f"ak{k}")
                    nc.scalar.activation(ak[:], img_t[:, c, :], AF.Sigmoid)
                    if k >= first_dirty:
                        tmp = tmpp.tile([128, OSH], f16, tag="tmp")
                        nc.scalar.activation(
                            tmp[:], im2_sb[:, k - first_dirty, :], AF.Sigmoid
                        )
                        nc.vector.tensor_add(ak[:], ak[:], tmp[:])
                    if k >= KCH - D3:
                        tmp = tmpp.tile([128, OSH], f16, tag="tmp")
                        nc.scalar.activation(
                            tmp[:], im3_sb[:, k - (KCH - D3), :], AF.Sigmoid
                        )
                        nc.vector.tensor_add(ak[:], ak[:], tmp[:])
                    aks[k] = ak
                    # phase-0 matmuls inline with chunk production
                    q0, qn = QPH[0]
                    for oc in range(OC):
                        for off in range(q0, q0 + qn, 512):
                            ln = min(512, q0 + qn - off)
                            nc.tensor.matmul(
                                pss[oc][:, off : off + ln],
                                ak[:, bass.ts(oc, 128)],
                                xt_t[:, c, off : off + ln],
                                start=(k == 0),
                                stop=(k == KCH - 1),
                            )

            nbia_sb = constp.tile([128, OC], f32)
            nc.scalar.mul(nbia_sb[:], bia_sb[:], -1.0)

            out_engines = [nc.sync, nc.gpsimd, nc.sync, nc.gpsimd]

            def epilogue(qi, oc, eng):
                q0, qn = QPH[qi]
                ob = obp.tile([128, qn], f16, tag="ob", name=f"ob{qi}_{oc}")
                nc.scalar.activation(
                    ob[:],
                    pss[oc][:, q0 : q0 + qn],
                    AF.Sigmoid,
                    bias=nbia_sb[:, oc : oc + 1],
                    scale=1.0,
                )
                eng.dma_start(
                    out_d.ap()[128 * oc : 128 * (oc + 1), q0 : q0 + qn], ob[:]
                )

            # phase-0 epilogue (overlaps phase-1 matmuls)
            for oc in range(OC):
                epilogue(0, oc, out_engines[oc])

            # phase 1: all chunks resident; epilogue overlaps phase 2
            q0, qn = QPH[1]
            for k in range(KCH):
                xt_t, c = xts[k]
                for oc in range(OC):
                    nc.tensor.matmul(
                        pss[oc][:, q0 : q0 + qn],
                        aks[k][:, bass.ts(oc, 128)],
                        xt_t[:, c, q0 : q0 + qn],
                        start=(k == 0),
                        stop=(k == KCH - 1),
                    )
            for oc in range(OC):
                epilogue(1, oc, out_engines[oc])

            # phase 2: per-oc chunk loops so each oc's epilogue overlaps the
            # next oc's matmuls; spread final DMAs across queues
            q0, qn = QPH[2]
            for oc in range(OC):
                for k in range(KCH):
                    xt_t, c = xts[k]
                    nc.tensor.matmul(
                        pss[oc][:, q0 : q0 + qn],
                        aks[k][:, bass.ts(oc, 128)],
                        xt_t[:, c, q0 : q0 + qn],
                        start=(k == 0),
                        stop=(k == KCH - 1),
                    )
                epilogue(2, oc, out_engines[oc])

    nc.compile()
    _CACHE["nc"] = nc
    return nc


def _host_prep(x, input_selection, weights, biases):
    """Index/layout-only host prep. Returns per-core input maps."""
    x = np.asarray(x, dtype=np.float32)
    sel = np.asarray(input_selection, dtype=np.int32)
    w = np.asarray(weights, dtype=np.float32)
    b = np.asarray(biases, dtype=np.float32)

    xT = np.ascontiguousarray(x.T).astype(np.float16)  # [I, B]

    in_maps = []
    for c in range(NCORES):
        sl = slice(c * OSH, (c + 1) * OSH)
        sel_c = sel[sl]          # [OSH, POP]
        w_c = w[sl]              # [OSH, POP]
        b_c = b[sl]              # [OSH]

        i_flat = sel_c.ravel().astype(np.int64)
        o_flat = np.repeat(np.arange(OSH, dtype=np.int64), POP)
        w_flat = w_c.ravel()
        order = np.lexsort((o_flat, i_flat))
        i_s, o_s, w_s = i_flat[order], o_flat[order], w_flat[order]

        # rank of each entry within its (i, o) cell: 0 = first, 1 = dup, ...
        same = np.zeros(i_s.size, dtype=bool)
        same[1:] = (i_s[1:] == i_s[:-1]) & (o_s[1:] == o_s[:-1])
        rank = np.zeros(i_s.size, dtype=np.int64)
        run = 0
        # vectorized run-length rank via cumsum trick
        idx = np.arange(i_s.size)
        start_idx = np.where(~same, idx, 0)
        np.maximum.accumulate(start_idx, out=start_idx)
        rank = idx - start_idx
        maxmult = rank.max() + 1
        if maxmult > 3:
            raise ValueError(f"unsupported cell multiplicity {maxmult}")

        # per-row dirtiness: max rank of any entry in row i
        row_rank = np.zeros(I, dtype=np.int64)
        np.maximum.at(row_rank, i_s, rank)
        rows3 = np.where(row_rank >= 2)[0]
        rows2 = np.where(row_rank == 1)[0]
        rows_clean = np.where(row_rank == 0)[0]
        n_dirty_cap = D_DIRTY * 128
        if len(rows2) + len(rows3) > n_dirty_cap:
            raise ValueError(
                f"dirty rows {len(rows2)+len(rows3)} exceed {n_dirty_cap}"
            )
        if len(rows3) > D3 * 128:
            raise ValueError(f"mult-3 rows {len(rows3)} exceed {D3*128}")
        # perm: clean rows, then mult-2 rows, then mult-3 rows at the very end
        n23 = len(rows2) + len(rows3)
        perm = np.concatenate(
            [rows_clean[: I - n23 - 0], rows2, rows3]
        )
        # clean rows fill the front; dirty rows must sit in the last chunks:
        # place rows3 at the absolute end, rows2 right before them.
        perm = np.concatenate([rows_clean, rows2, rows3])
        assert perm.size == I
        pos = np.empty(I, dtype=np.int64)
        pos[perm] = np.arange(I)

        # images
        img1 = np.full((I, OSH), NEG, np.float16)
        img2 = np.full((n_dirty_cap, OSH), NEG, np.float16)
        img3 = np.full((D3 * 128, OSH), NEG, np.float16)
        p_s = pos[i_s]
        r0 = rank == 0
        img1[p_s[r0], o_s[r0]] = w_s[r0].astype(np.float16)
        d_base = I - n_dirty_cap
        r1 = rank == 1
        assert (p_s[r1] >= d_base).all()
        img2[p_s[r1] - d_base, o_s[r1]] = w_s[r1].astype(np.float16)
        t_base = I - D3 * 128
        r2 = rank == 2
        assert (p_s[r2] >= t_base).all()
        img3[p_s[r2] - t_base, o_s[r2]] = w_s[r2].astype(np.float16)

        xT_c = np.ascontiguousarray(xT[perm])

        bias_t = np.ascontiguousarray(b_c.reshape(OC, 128).T)  # [128, OC]

        in_maps.append(
            {
                "xT": xT_c,
                "img1": img1,
                "img2": img2,
                "img3": img3,
                "bias": bias_t,
            }
        )
    return in_maps


def kernel(x, input_selection, weights, biases):
    nc = _build()
    in_maps = _host_prep(x, input_selection, weights, biases)
    res = bass_utils.run_bass_kernel_spmd(nc, in_maps, core_ids=list(range(NCORES)))
    outT = np.concatenate(
        [np.asarray(res.results[c]["outT"]) for c in range(NCORES)], axis=0
    )  # [O, B] f16
    return np.ascontiguousarray(outT.T.astype(np.float32))  # [B, O]


# revision 20
# speedup vs baseline: 1.1607x; 1.1607x over previous
"""Trainium2 Bass kernel for nn_PopcntLayer (segment_reduce).

Computation: out[b,o] = sigmoid( sum_p x[b, sel[o,p]] * sigmoid(w[o,p]) - bias[o] )
 with x [1024, 4096] f32, sel [4096, 64] i32, w [4096, 64] f32, bias [4096] f32.

Strategy (output-width sharded across 8 cores, 512 outputs each):
  out = sigmoid(x @ A - bias) where A[i, o] = sum_{p: sel[o,p]=i} sigmoid(w[o,p])
  is a sparse (64 nnz per column) matrix applied as a dense [4096, 512] f16
  operand on the PE.

Host prep is index/layout-only: raw weights are scattered into a dense
[4096, 512] f16 "image" (empty cells = -30000, so sigmoid maps them to 0),
and input rows are permuted per-core so rows containing duplicate (i, o)
cells land in the last D_DIRTY chunks; duplicates' 2nd/3rd weights go to
small overflow images (im2/im3). Each 128-row chunk of the permuted xT and
the image are packed side by side into one DRAM row block, so a single DMA
stream supplies both matmul operands in order (no cross-queue skew). On
device, ACT applies sigmoid to each image chunk to produce the A-chunk
(overflow layers are sigmoided and added with DVE), and the PE accumulates
out.T[o,b] += A_k.T @ xT_k.

The batch is processed in staggered phases (QPH) so the psum -> sigmoid ->
DMA epilogues of early phases overlap the matmul stream of later ones.
Junk matmuls keep the PE p-state ramp warm until the first A-chunk is
ready; supply DMAs ride the SP queue, side tensors and early-phase outputs
ride the Pool/SWDGE queue.

The kernel computes out.T per core ([512, 1024] f16); host concatenates and
transposes back.
"""

import os
import sys

for _p in ("/opt/trn_rl_repo", "/root/.axon_site/_ro/trn_rl_repo"):
    if os.path.isdir(_p) and _p not in sys.path:
        sys.path.append(_p)

import numpy as np

import concourse.bass as bass
import concourse.tile as tile
import concourse.mybir as mybir
from concourse import bacc
from concourse import bass_utils

B = 1024          # batch
I = 4096          # input width
O = 4096          # output width
POP = 64          # popcount width
NCORES = 8
OSH = O // NCORES     # 512 outputs per core
KCH = I // 128        # 32 contraction chunks
OC = OSH // 128       # 4 output chunks per core
D_DIRTY = 3           # trailing chunks holding rows with duplicate cells
D3 = 1                # trailing chunks holding multiplicity-3 rows
NEG = -30000.0        # sigmoid(NEG) == 0 in f32/f16
PKW = B + OSH         # packed row width (xT | img)

# batch phases: staggered so epilogues overlap later phases' matmuls
QPH = [(0, 768), (768, 192), (960, 64)]
# junk warmup matmuls: cover the PE until the first A chunk lands (they run
# in the cold p-state window anyway, so they waste nothing useful)
N_JUNK = 15
JUNK_F = 256
# DMA chunk batching: leading singles for a fast head, then pairs
BATCH_PLAN = [1] * 6 + [2] * 13
USE_TRIGGER_TAIL = os.environ.get("KTRIG", "1") == "1"
SIDE_AFTER_BATCH = 9  # inject the side DMA into the SP stream at this batch
FIRST_DIRTY = 24      # chunk index of first duplicate-overflow chunk
D3_CHUNK = FIRST_DIRTY + D_DIRTY - 1  # mult-3 rows: tail of the dirty window

_CACHE = {}


def _build():
    """Build + compile the (SPMD, identical on all cores) Bass program."""
    if "nc" in _CACHE:
        return _CACHE["nc"]
    f32 = mybir.dt.float32
    f16 = mybir.dt.float16
    AF = mybir.ActivationFunctionType

    nc = bacc.Bacc("TRN2", debug=False)
    pk_d = nc.dram_tensor("pk", [I, PKW], f16, kind="ExternalInput")
    sd_d = nc.dram_tensor(
        "side", [128, (D_DIRTY + D3) * OSH + 2 * OC + 8], mybir.dt.int16,
        kind="ExternalInput"
    )
    out_d = nc.dram_tensor("outT", [OSH, B], f16, kind="ExternalOutput")

    assert sum(BATCH_PLAN) == KCH
    batches = []
    k = 0
    for n in BATCH_PLAN:
        batches.append(list(range(k, k + n)))
        k += n

    with tile.TileContext(nc) as tc:
        with (
            tc.tile_pool(name="const", bufs=1) as constp,
            tc.tile_pool(name="pk", bufs=len(batches)) as pkp,
            tc.tile_pool(name="ak", bufs=KCH) as akp,
            tc.tile_pool(name="tmp", bufs=2) as tmpp,
            tc.tile_pool(name="ps", bufs=1, space="PSUM") as psp,
            tc.tile_pool(name="ob", bufs=4) as obp,
        ):
            pss = [
                psp.tile([128, B], f32, tag=f"ps{oc}", name=f"ps{oc}")
                for oc in range(OC)
            ]

            # Warm tile + junk matmuls: PE busy from ~1.5us so the p-state
            # ramp completes by the time real matmuls start.
            warm = constp.tile([128, JUNK_F], f16)
            nc.vector.memset(warm[:], 0.0)
            for _ in range(N_JUNK):
                nc.tensor.matmul(
                    pss[0][:, 0:JUNK_F], warm[:, 0:128], warm[:],
                    start=True, stop=True,
                )

            # side tensor (im2 | im3 | bias | scatter idxs): DMA'd mid-stream
            SDW = (D_DIRTY + D3) * OSH + 2 * OC + 8
            sd_sb = constp.tile([128, SDW], mybir.dt.int16, tag="side",
                                name="side")
            bia_f32 = sd_sb[:, (D_DIRTY + D3) * OSH : (D_DIRTY + D3) * OSH
                            + 2 * OC].bitcast(f32)
            idn_i16 = sd_sb[:, SDW - 8 : SDW]

            # final-piece output tile + completion plumbing: the scatter-add
            # descriptors are PREPARED mid-kernel and TRIGGERED right after
            # the last epilogue sigmoid, skipping the DGE issue latency that
            # would otherwise sit on the kernel tail
            ob_last = constp.tile([128, 1, QPH[2][1]], f16, tag="obl",
                                  name="ob_last")
            if USE_TRIGGER_TAIL:
                dma_done = nc.alloc_semaphore("dma_done")
                nc.vector.memset(ob_last[:], 0.0)

            pks = [None] * KCH      # (tile, idx) per chunk
            aks = [None] * KCH

            for bi, bk in enumerate(batches):
                n = len(bk)
                k0 = bk[0]
                pk_t = pkp.tile([128, n, PKW], f16, tag="pk", name=f"pk{bi}")
                if bi == 0:
                    # split chunk 0: image part first so its sigmoid starts
                    # while the batch columns are still in flight
                    nc.sync.dma_start(
                        pk_t[:, 0, B:PKW], pk_d.ap()[0:128, B:PKW]
                    )
                    nc.sync.dma_start(pk_t[:, 0, 0:B], pk_d.ap()[0:128, 0:B])
                else:
                    nc.sync.dma_start(
                        pk_t[:],
                        pk_d.ap()[k0 * 128 : (k0 + n) * 128, :].rearrange(
                            "(c p) w -> p c w", p=128
                        ),
                    )
                if bi == SIDE_AFTER_BATCH:
                    # side tensor rides the SP stream here: late enough not
                    # to delay head chunks, early enough for chunk FIRST_DIRTY
                    nc.sync.dma_start(sd_sb[:], sd_d.ap())
                for c, kk in enumerate(bk):
                    pks[kk] = (pk_t, c)
                    ak = akp.tile([128, OSH], f16, tag="ak", name=f"ak{kk}")
                    nc.scalar.activation(
                        ak[:], pk_t[:, c, B : B + OSH], AF.Sigmoid
                    )
                    if FIRST_DIRTY <= kk < FIRST_DIRTY + D_DIRTY:
                        d = kk - FIRST_DIRTY
                        tmp = tmpp.tile([128, OSH], f16, tag="tmp")
                        nc.scalar.activation(
                            tmp[:],
                            sd_sb[:, d * OSH : (d + 1) * OSH].bitcast(f16),
                            AF.Sigmoid,
                        )
                        nc.vector.tensor_add(ak[:], ak[:], tmp[:])
                    if D3_CHUNK <= kk < D3_CHUNK + D3:
                        tmp = tmpp.tile([128, OSH], f16, tag="tmp")
                        nc.scalar.activation(
                            tmp[:],
                            sd_sb[:, D_DIRTY * OSH : (D_DIRTY + 1) * OSH]
                            .bitcast(f16),
                            AF.Sigmoid,
                        )
                        nc.vector.tensor_add(ak[:], ak[:], tmp[:])
                    aks[kk] = ak
                    # phase-0 matmuls inline with chunk production
                    q0, qn = QPH[0]
                    for oc in range(OC):
                        for off in range(0, qn, 512):
                            ln = min(512, qn - off)
                            nc.tensor.matmul(
                                pss[oc][:, q0 + off : q0 + off + ln],
                                ak[:, bass.ts(oc, 128)],
                                pk_t[:, c, q0 + off : q0 + off + ln],
                                start=(kk == 0),
                                stop=(kk == KCH - 1),
                            )

            nbia_sb = constp.tile([128, OC], f32)
            nc.scalar.mul(nbia_sb[:], bia_f32, -1.0)

            def epilogue(qi, oc, eng):
                q0, qn = QPH[qi]
                ob = obp.tile([128, qn], f16, tag=f"ob{qi}", name=f"ob{qi}_{oc}")
                nc.scalar.activation(
                    ob[:],
                    pss[oc][:, q0 : q0 + qn],
                    AF.Sigmoid,
                    bias=nbia_sb[:, oc : oc + 1],
                    scale=1.0,
                )
                eng.dma_start(
                    out_d.ap()[128 * oc : 128 * (oc + 1), q0 : q0 + qn], ob[:]
                )

            def prep_scatter(qi, oc, src_tile):
                q_0, q_n = QPH[qi]
                return nc.gpsimd.dma_scatter_add(
                    out_d.ap()[128 * oc : 128 * (oc + 1), q_0 : q_0 + q_n],
                    src_tile[:],
                    idn_i16,
                    128,
                    128,
                    q_n,
                    elem_step=B,
                    prepare_only=True,
                    sem=dma_done,
                )

            # early-prepared scatter for the q1-oc3 output piece (ring pos 0)
            ob_l1 = constp.tile([128, 1, QPH[1][1]], f16, tag="obl1",
                                name="ob_l1")
            if USE_TRIGGER_TAIL:
                nc.vector.memset(ob_l1[:], 0.0)
                prep_scatter(1, OC - 1, ob_l1)

            # phase-0 epilogue (overlaps phase-1 matmuls)
            for oc in range(OC):
                epilogue(0, oc, nc.sync)

            # phases 1+2 interleaved per-oc so every epilogue drains while
            # later ocs' matmuls still stream; the two tail pieces (q1-oc3,
            # q2-oc3) go out via prepared scatter-adds fired by trigger_dma
            for oc in range(OC):
                for qi in (1, 2):
                    q0, qn = QPH[qi]
                    for kk in range(KCH):
                        pk_t, c = pks[kk]
                        nc.tensor.matmul(
                            pss[oc][:, q0 : q0 + qn],
                            aks[kk][:, bass.ts(oc, 128)],
                            pk_t[:, c, q0 : q0 + qn],
                            start=(kk == 0),
                            stop=(kk == KCH - 1),
                        )
                    if oc < OC - 1 or not USE_TRIGGER_TAIL:
                        epilogue(qi, oc, nc.sync)
                    elif qi == 1:
                        nc.scalar.activation(
                            ob_l1[:, 0, :],
                            pss[oc][:, q0 : q0 + qn],
                            AF.Sigmoid,
                            bias=nbia_sb[:, oc : oc + 1],
                            scale=1.0,
                        )
                        nc.gpsimd.trigger_dma(count=None)
                        # JIT-prep the q2-oc3 scatter now (after the trigger
                        # above, so it isn't swept by it); its desc-gen runs
                        # while the q2-oc3 matmuls stream
                        prep_scatter(2, OC - 1, ob_last)
                    else:
                        nc.scalar.activation(
                            ob_last[:, 0, :],
                            pss[oc][:, q0 : q0 + qn],
                            AF.Sigmoid,
                            bias=nbia_sb[:, oc : oc + 1],
                            scale=1.0,
                        )
                        nc.gpsimd.trigger_dma(count=None)
                        # consume both prepared DMAs' completion credits so
                        # the program provably finishes after the writes land
                        nc.sync.wait_ge(dma_done, 32)

    # Patch the prepared scatter-add's completion sem to the DMASW lane sem
    # the tile end-gate actually waits on (sem= preempts slot 0, but the
    # drain gate is built against the lane clock).
    # Tile builds its end-of-kernel DMA gate against the DMASW lane clocks,
    # but prepared DMAs credit the user sem (dma_done) instead; the explicit
    # wait_ge(dma_done) emitted above is the real completion gate, so strip
    # the never-credited lane waits to avoid a deadlock.
    fn = nc.m.functions[0]
    total = 0 if USE_TRIGGER_TAIL else 32
    for blk in fn.blocks:
        for inst in blk.instructions:
            if USE_TRIGGER_TAIL and type(inst).__name__ == "InstDMAScatterAddAnt":
                u0 = inst.sync_info.on_update[0]
                assert (u0.ant_name or "") == "dma_done", u0
                total += u0.update_value
            si = getattr(inst, "sync_info", None)
            if si is not None and any(
                (w.ant_name or "").startswith("DMASW") for w in si.on_wait or []
            ):
                si.on_wait = [
                    w
                    for w in si.on_wait
                    if not (w.ant_name or "").startswith("DMASW")
                ]
    assert total == 32, total

    nc.compile()
    _CACHE["nc"] = nc
    return nc


def _host_prep(x, input_selection, weights, biases):
    """Index/layout-only host prep. Returns per-core input maps."""
    x = np.asarray(x, dtype=np.float32)
    sel = np.asarray(input_selection, dtype=np.int32)
    w = np.asarray(weights, dtype=np.float32)
    b = np.asarray(biases, dtype=np.float32)

    xT = np.ascontiguousarray(x.T).astype(np.float16)  # [I, B]

    in_maps = []
    for c in range(NCORES):
        sl = slice(c * OSH, (c + 1) * OSH)
        sel_c = sel[sl]          # [OSH, POP]
        w_c = w[sl]              # [OSH, POP]
        b_c = b[sl]              # [OSH]

        i_flat = sel_c.ravel().astype(np.int64)
        o_flat = np.repeat(np.arange(OSH, dtype=np.int64), POP)
        w_flat = w_c.ravel()
        order = np.lexsort((o_flat, i_flat))
        i_s, o_s, w_s = i_flat[order], o_flat[order], w_flat[order]

        # rank of each entry within its (i, o) cell: 0 = first, 1 = dup, ...
        same = np.zeros(i_s.size, dtype=bool)
        same[1:] = (i_s[1:] == i_s[:-1]) & (o_s[1:] == o_s[:-1])
        idx = np.arange(i_s.size)
        start_idx = np.where(~same, idx, 0)
        np.maximum.accumulate(start_idx, out=start_idx)
        rank = idx - start_idx
        maxmult = rank.max() + 1
        if maxmult > 3:
            raise ValueError(f"unsupported cell multiplicity {maxmult}")

        # per-row dirtiness: max rank of any entry in row i
        row_rank = np.zeros(I, dtype=np.int64)
        np.maximum.at(row_rank, i_s, rank)
        rows3 = np.where(row_rank >= 2)[0]
        rows2 = np.where(row_rank == 1)[0]
        rows_clean = np.where(row_rank == 0)[0]
        n_dirty_cap = D_DIRTY * 128
        if len(rows2) + len(rows3) > n_dirty_cap:
            raise ValueError(
                f"dirty rows {len(rows2)+len(rows3)} exceed {n_dirty_cap}"
            )
        if len(rows3) > D3 * 128:
            raise ValueError(f"mult-3 rows {len(rows3)} exceed {D3*128}")
        # perm: dirty rows sit in chunks FIRST_DIRTY..FIRST_DIRTY+D_DIRTY-1,
        # mult-3 rows in chunk D3_CHUNK; clean rows pad and fill the rest
        n2, n3 = len(rows2), len(rows3)
        n2_cap = n_dirty_cap - D3 * 128   # rows2 live before the mult-3 rows
        if n2 > n2_cap:
            raise ValueError(f"mult-2 rows {n2} exceed {n2_cap}")
        pad2 = rows_clean[: n2_cap - n2]
        pad3 = rows_clean[n2_cap - n2 : n2_cap - n2 + D3 * 128 - n3]
        rest = rows_clean[n2_cap - n2 + D3 * 128 - n3 :]
        d_lo = FIRST_DIRTY * 128
        perm = np.concatenate(
            [rest[:d_lo], rows2, pad2, rows3, pad3, rest[d_lo:]]
        )
        assert perm.size == I
        pos = np.empty(I, dtype=np.int64)
        pos[perm] = np.arange(I)

        # images
        img1 = np.full((I, OSH), NEG, np.float16)
        img2 = np.full((n_dirty_cap, OSH), NEG, np.float16)
        img3 = np.full((D3 * 128, OSH), NEG, np.float16)
        p_s = pos[i_s]
        r0 = rank == 0
        img1[p_s[r0], o_s[r0]] = w_s[r0].astype(np.float16)
        d_base = FIRST_DIRTY * 128
        r1 = rank == 1
        assert (p_s[r1] >= d_base).all()
        img2[p_s[r1] - d_base, o_s[r1]] = w_s[r1].astype(np.float16)
        t_base = D3_CHUNK * 128
        r2 = rank == 2
        assert (p_s[r2] >= t_base).all()
        img3[p_s[r2] - t_base, o_s[r2]] = w_s[r2].astype(np.float16)

        # packed supply stream: [xT | img1] per permuted row
        pk = np.concatenate([xT[perm], img1], axis=1)  # [I, PKW] f16
        assert pk.shape == (I, PKW)

        # side tensor: [im2 (3 chunks) | im3 | bias-as-f16-words] per partition
        im2_t = np.ascontiguousarray(
            img2.reshape(D_DIRTY, 128, OSH).transpose(1, 0, 2).reshape(
                128, D_DIRTY * OSH
            )
        )
        im3_t = img3  # [128, OSH]
        bias_t = np.ascontiguousarray(b_c.reshape(OC, 128).T)  # [128, OC] f32
        bias_w = bias_t.astype(np.float32).view(np.float16)    # [128, 2*OC]
        idn = np.full((128, 8), -1, np.int16)
        idn[:16, :] = np.arange(8)[None, :] * 16 + np.arange(16)[:, None]
        side = np.concatenate(
            [
                im2_t.view(np.int16),
                im3_t.view(np.int16),
                bias_w.view(np.int16),
                idn,
            ],
            axis=1,
        )

        in_maps.append({"pk": pk, "side": side})
    return in_maps


def kernel(x, input_selection, weights, biases):
    nc = _build()
    in_maps = _host_prep(x, input_selection, weights, biases)
    res = bass_utils.run_bass_kernel_spmd(nc, in_maps, core_ids=list(range(NCORES)))
    outT = np.concatenate(
        [np.asarray(res.results[c]["outT"]) for c in range(NCORES)], axis=0
    )  # [O, B] f16
    return np.ascontiguousarray(outT.T.astype(np.float32))  # [B, O]


# revision 25
# speedup vs baseline: 1.1838x; 1.0199x over previous
"""Trainium2 Bass kernel for nn_PopcntLayer (segment_reduce).

Computation: out[b,o] = sigmoid( sum_p x[b, sel[o,p]] * sigmoid(w[o,p]) - bias[o] )
 with x [1024, 4096] f32, sel [4096, 64] i32, w [4096, 64] f32, bias [4096] f32.

Strategy (output-width sharded across 8 cores, 512 outputs each):
  out = sigmoid(x @ A - bias) where A[i, o] = sum_{p: sel[o,p]=i} sigmoid(w[o,p])
  is a sparse (64 nnz per column) matrix applied as a dense [4096, 512] f16
  operand on the PE.

Host prep is index/layout-only: raw weights are scattered into a dense
[4096, 512] f16 "image" (empty cells = -30000, so sigmoid maps them to 0),
and input rows are permuted per-core so rows containing duplicate (i, o)
cells land in the last D_DIRTY chunks; duplicates' 2nd/3rd weights go to
small overflow images (im2/im3). Each 128-row chunk of the permuted xT and
the image are packed side by side into one DRAM row block, so a single DMA
stream supplies both matmul operands in order (no cross-queue skew). On
device, ACT applies sigmoid to each image chunk to produce the A-chunk
(overflow layers are sigmoided and added with DVE), and the PE accumulates
out.T[o,b] += A_k.T @ xT_k.

The batch is processed in staggered phases (QPH) so the psum -> sigmoid ->
DMA epilogues of early phases overlap the matmul stream of later ones.
Junk matmuls keep the PE p-state ramp warm until the first A-chunk is
ready; supply DMAs ride the SP queue, side tensors and early-phase outputs
ride the Pool/SWDGE queue.

The kernel computes out.T per core ([512, 1024] f16); host concatenates and
transposes back.
"""

import os
import sys

for _p in ("/opt/trn_rl_repo", "/root/.axon_site/_ro/trn_rl_repo"):
    if os.path.isdir(_p) and _p not in sys.path:
        sys.path.append(_p)

import numpy as np

import concourse.bass as bass
import concourse.tile as tile
import concourse.mybir as mybir
from concourse import bacc
from concourse import bass_utils

B = 1024          # batch
I = 4096          # input width
O = 4096          # output width
POP = 64          # popcount width
NCORES = 8
OSH = O // NCORES     # 512 outputs per core
KCH = I // 128        # 32 contraction chunks
OC = OSH // 128       # 4 output chunks per core
D_DIRTY = 3           # trailing chunks holding rows with duplicate cells
D3 = 1                # trailing chunks holding multiplicity-3 rows
NEG = -30000.0        # sigmoid(NEG) == 0 in f32/f16
PKW = B + OSH         # packed row width (xT | img)

# batch phases: staggered so epilogues overlap later phases' matmuls
QPH = [(0, 768), (768, 192), (960, 64)]
# junk warmup matmuls: cover the PE until the first A chunk lands (they run
# in the cold p-state window anyway, so they waste nothing useful)
N_JUNK = 15
JUNK_F = 256
# DMA chunk batching: leading singles for a fast head, then pairs
BATCH_PLAN = [1] * 6 + [2] * 13
USE_TRIGGER_TAIL = os.environ.get("KTRIG", "0") == "1"
SIDE_AFTER_BATCH = 9  # inject the side DMA into the SP stream at this batch
FIRST_DIRTY = 24      # chunk index of first duplicate-overflow chunk
D3_CHUNK = FIRST_DIRTY + D_DIRTY - 1  # mult-3 rows: tail of the dirty window

_CACHE = {}


def _build():
    """Build + compile the (SPMD, identical on all cores) Bass program."""
    if "nc" in _CACHE:
        return _CACHE["nc"]
    f32 = mybir.dt.float32
    f16 = mybir.dt.float16
    AF = mybir.ActivationFunctionType

    nc = bacc.Bacc("TRN2", debug=False)
    pk_d = nc.dram_tensor("pk", [I, PKW], f16, kind="ExternalInput")
    sd_d = nc.dram_tensor(
        "side", [128, (D_DIRTY + D3) * OSH + 2 * OC + 8], mybir.dt.int16,
        kind="ExternalInput"
    )
    out_d = nc.dram_tensor("outT", [OSH, B], f16, kind="ExternalOutput")

    assert sum(BATCH_PLAN) == KCH
    batches = []
    k = 0
    for n in BATCH_PLAN:
        batches.append(list(range(k, k + n)))
        k += n

    with tile.TileContext(nc) as tc:
        with (
            tc.tile_pool(name="const", bufs=1) as constp,
            tc.tile_pool(name="pk", bufs=len(batches)) as pkp,
            tc.tile_pool(name="ak", bufs=KCH) as akp,
            tc.tile_pool(name="tmp", bufs=2) as tmpp,
            tc.tile_pool(name="ps", bufs=1, space="PSUM") as psp,
            tc.tile_pool(name="ob", bufs=4) as obp,
        ):
            pss = [
                psp.tile([128, B], f32, tag=f"ps{oc}", name=f"ps{oc}")
                for oc in range(OC)
            ]

            # Warm tile + junk matmuls: PE busy from ~1.5us so the p-state
            # ramp completes by the time real matmuls start.
            warm = constp.tile([128, JUNK_F], f16)
            nc.vector.memset(warm[:], 0.0)
            for _ in range(N_JUNK):
                nc.tensor.matmul(
                    pss[0][:, 0:JUNK_F], warm[:, 0:128], warm[:],
                    start=True, stop=True,
                )

            # side tensor (im2 | im3 | bias | scatter idxs): DMA'd mid-stream
            SDW = (D_DIRTY + D3) * OSH + 2 * OC + 8
            sd_sb = constp.tile([128, SDW], mybir.dt.int16, tag="side",
                                name="side")
            bia_f32 = sd_sb[:, (D_DIRTY + D3) * OSH : (D_DIRTY + D3) * OSH
                            + 2 * OC].bitcast(f32)
            idn_i16 = sd_sb[:, SDW - 8 : SDW]

            # final-piece output tile + completion plumbing: the scatter-add
            # descriptors are PREPARED mid-kernel and TRIGGERED right after
            # the last epilogue sigmoid, skipping the DGE issue latency that
            # would otherwise sit on the kernel tail
            ob_last = constp.tile([128, 1, QPH[2][1]], f16, tag="obl",
                                  name="ob_last")
            if USE_TRIGGER_TAIL:
                dma_done = nc.alloc_semaphore("dma_done")
                nc.vector.memset(ob_last[:], 0.0)

            pks = [None] * KCH      # (tile, idx) per chunk
            aks = [None] * KCH

            for bi, bk in enumerate(batches):
                n = len(bk)
                k0 = bk[0]
                pk_t = pkp.tile([128, n, PKW], f16, tag="pk", name=f"pk{bi}")
                if bi == 0:
                    # split chunk 0: image part first so its sigmoid starts
                    # while the batch columns are still in flight
                    nc.sync.dma_start(
                        pk_t[:, 0, B:PKW], pk_d.ap()[0:128, B:PKW]
                    )
                    nc.sync.dma_start(pk_t[:, 0, 0:B], pk_d.ap()[0:128, 0:B])
                else:
                    nc.sync.dma_start(
                        pk_t[:],
                        pk_d.ap()[k0 * 128 : (k0 + n) * 128, :].rearrange(
                            "(c p) w -> p c w", p=128
                        ),
                    )
                if bi == SIDE_AFTER_BATCH:
                    # side tensor rides the SP stream here: late enough not
                    # to delay head chunks, early enough for chunk FIRST_DIRTY
                    nc.sync.dma_start(sd_sb[:], sd_d.ap())
                for c, kk in enumerate(bk):
                    pks[kk] = (pk_t, c)
                    ak = akp.tile([128, OSH], f16, tag="ak", name=f"ak{kk}")
                    nc.scalar.activation(
                        ak[:], pk_t[:, c, B : B + OSH], AF.Sigmoid
                    )
                    if FIRST_DIRTY <= kk < FIRST_DIRTY + D_DIRTY:
                        d = kk - FIRST_DIRTY
                        tmp = tmpp.tile([128, OSH], f16, tag="tmp")
                        nc.scalar.activation(
                            tmp[:],
                            sd_sb[:, d * OSH : (d + 1) * OSH].bitcast(f16),
                            AF.Sigmoid,
                        )
                        nc.vector.tensor_add(ak[:], ak[:], tmp[:])
                    if D3_CHUNK <= kk < D3_CHUNK + D3:
                        tmp = tmpp.tile([128, OSH], f16, tag="tmp")
                        nc.scalar.activation(
                            tmp[:],
                            sd_sb[:, D_DIRTY * OSH : (D_DIRTY + 1) * OSH]
                            .bitcast(f16),
                            AF.Sigmoid,
                        )
                        nc.vector.tensor_add(ak[:], ak[:], tmp[:])
                    aks[kk] = ak
                    # phase-0 matmuls inline with chunk production
                    q0, qn = QPH[0]
                    for oc in range(OC):
                        for off in range(0, qn, 512):
                            ln = min(512, qn - off)
                            nc.tensor.matmul(
                                pss[oc][:, q0 + off : q0 + off + ln],
                                ak[:, bass.ts(oc, 128)],
                                pk_t[:, c, q0 + off : q0 + off + ln],
                                start=(kk == 0),
                                stop=(kk == KCH - 1),
                            )

            nbia_sb = constp.tile([128, OC], f32)
            nc.scalar.mul(nbia_sb[:], bia_f32, -1.0)

            def epilogue(qi, oc, eng):
                q0, qn = QPH[qi]
                ob = obp.tile([128, qn], f16, tag=f"ob{qi}", name=f"ob{qi}_{oc}")
                nc.scalar.activation(
                    ob[:],
                    pss[oc][:, q0 : q0 + qn],
                    AF.Sigmoid,
                    bias=nbia_sb[:, oc : oc + 1],
                    scale=1.0,
                )
                eng.dma_start(
                    out_d.ap()[128 * oc : 128 * (oc + 1), q0 : q0 + qn], ob[:]
                )

            def prep_scatter(qi, oc, src_tile):
                q_0, q_n = QPH[qi]
                return nc.gpsimd.dma_scatter_add(
                    out_d.ap()[128 * oc : 128 * (oc + 1), q_0 : q_0 + q_n],
                    src_tile[:],
                    idn_i16,
                    128,
                    128,
                    q_n,
                    elem_step=B,
                    prepare_only=True,
                    sem=dma_done,
                )

            # early-prepared scatter for the q1-oc3 output piece (ring pos 0)
            ob_l1 = constp.tile([128, 1, QPH[1][1]], f16, tag="obl1",
                                name="ob_l1")
            if USE_TRIGGER_TAIL:
                nc.vector.memset(ob_l1[:], 0.0)
                prep_scatter(1, OC - 1, ob_l1)

            # phase-0 epilogue (overlaps phase-1 matmuls)
            for oc in range(OC):
                epilogue(0, oc, nc.sync)

            # phases 1 then 2, each swept per-oc: an oc's phase-2 group
            # starts 3 blocks after its phase-1 epilogue, so the psum-tile
            # WAR edges (epilogue ACT -> next accumulation group) are slack
            for qi in (1, 2):
                for oc in range(OC):
                    q0, qn = QPH[qi]
                    for kk in range(KCH):
                        pk_t, c = pks[kk]
                        nc.tensor.matmul(
                            pss[oc][:, q0 : q0 + qn],
                            aks[kk][:, bass.ts(oc, 128)],
                            pk_t[:, c, q0 : q0 + qn],
                            start=(kk == 0),
                            stop=(kk == KCH - 1),
                        )
                    if oc < OC - 1 or not USE_TRIGGER_TAIL:
                        epilogue(qi, oc, nc.sync)
                    elif qi == 1:
                        nc.scalar.activation(
                            ob_l1[:, 0, :],
                            pss[oc][:, q0 : q0 + qn],
                            AF.Sigmoid,
                            bias=nbia_sb[:, oc : oc + 1],
                            scale=1.0,
                        )
                        nc.gpsimd.trigger_dma(count=None)
                        # JIT-prep the q2-oc3 scatter now (after the trigger
                        # above, so it isn't swept by it); its desc-gen runs
                        # while the q2-oc3 matmuls stream
                        prep_scatter(2, OC - 1, ob_last)
                    else:
                        nc.scalar.activation(
                            ob_last[:, 0, :],
                            pss[oc][:, q0 : q0 + qn],
                            AF.Sigmoid,
                            bias=nbia_sb[:, oc : oc + 1],
                            scale=1.0,
                        )
                        nc.gpsimd.trigger_dma(count=None)
                        # consume both prepared DMAs' completion credits so
                        # the program provably finishes after the writes land
                        nc.sync.wait_ge(dma_done, 32)

    # Patch the prepared scatter-add's completion sem to the DMASW lane sem
    # the tile end-gate actually waits on (sem= preempts slot 0, but the
    # drain gate is built against the lane clock).
    # Tile builds its end-of-kernel DMA gate against the DMASW lane clocks,
    # but prepared DMAs credit the user sem (dma_done) instead; the explicit
    # wait_ge(dma_done) emitted above is the real completion gate, so strip
    # the never-credited lane waits to avoid a deadlock.
    fn = nc.m.functions[0]
    total = 0 if USE_TRIGGER_TAIL else 32
    for blk in fn.blocks:
        for inst in blk.instructions:
            if USE_TRIGGER_TAIL and type(inst).__name__ == "InstDMAScatterAddAnt":
                u0 = inst.sync_info.on_update[0]
                assert (u0.ant_name or "") == "dma_done", u0
                total += u0.update_value
            si = getattr(inst, "sync_info", None)
            if si is not None and any(
                (w.ant_name or "").startswith("DMASW") for w in si.on_wait or []
            ):
                si.on_wait = [
                    w
                    for w in si.on_wait
                    if not (w.ant_name or "").startswith("DMASW")
                ]
    assert total == 32, total

    nc.compile()
    _CACHE["nc"] = nc
    return nc


def _host_prep(x, input_selection, weights, biases):
    """Index/layout-only host prep. Returns per-core input maps."""
    x = np.asarray(x, dtype=np.float32)
    sel = np.asarray(input_selection, dtype=np.int32)
    w = np.asarray(weights, dtype=np.float32)
    b = np.asarray(biases, dtype=np.float32)

    xT = np.ascontiguousarray(x.T).astype(np.float16)  # [I, B]

    in_maps = []
    for c in range(NCORES):
        sl = slice(c * OSH, (c + 1) * OSH)
        sel_c = sel[sl]          # [OSH, POP]
        w_c = w[sl]              # [OSH, POP]
        b_c = b[sl]              # [OSH]

        i_flat = sel_c.ravel().astype(np.int64)
        o_flat = np.repeat(np.arange(OSH, dtype=np.int64), POP)
        w_flat = w_c.ravel()
        order = np.lexsort((o_flat, i_flat))
        i_s, o_s, w_s = i_flat[order], o_flat[order], w_flat[order]

        # rank of each entry within its (i, o) cell: 0 = first, 1 = dup, ...
        same = np.zeros(i_s.size, dtype=bool)
        same[1:] = (i_s[1:] == i_s[:-1]) & (o_s[1:] == o_s[:-1])
        idx = np.arange(i_s.size)
        start_idx = np.where(~same, idx, 0)
        np.maximum.accumulate(start_idx, out=start_idx)
        rank = idx - start_idx
        maxmult = rank.max() + 1
        if maxmult > 3:
            raise ValueError(f"unsupported cell multiplicity {maxmult}")

        # per-row dirtiness: max rank of any entry in row i
        row_rank = np.zeros(I, dtype=np.int64)
        np.maximum.at(row_rank, i_s, rank)
        rows3 = np.where(row_rank >= 2)[0]
        rows2 = np.where(row_rank == 1)[0]
        rows_clean = np.where(row_rank == 0)[0]
        n_dirty_cap = D_DIRTY * 128
        if len(rows2) + len(rows3) > n_dirty_cap:
            raise ValueError(
                f"dirty rows {len(rows2)+len(rows3)} exceed {n_dirty_cap}"
            )
        if len(rows3) > D3 * 128:
            raise ValueError(f"mult-3 rows {len(rows3)} exceed {D3*128}")
        # perm: dirty rows sit in chunks FIRST_DIRTY..FIRST_DIRTY+D_DIRTY-1,
        # mult-3 rows in chunk D3_CHUNK; clean rows pad and fill the rest
        n2, n3 = len(rows2), len(rows3)
        n2_cap = n_dirty_cap - D3 * 128   # rows2 live before the mult-3 rows
        if n2 > n2_cap:
            raise ValueError(f"mult-2 rows {n2} exceed {n2_cap}")
        pad2 = rows_clean[: n2_cap - n2]
        pad3 = rows_clean[n2_cap - n2 : n2_cap - n2 + D3 * 128 - n3]
        rest = rows_clean[n2_cap - n2 + D3 * 128 - n3 :]
        d_lo = FIRST_DIRTY * 128
        perm = np.concatenate(
            [rest[:d_lo], rows2, pad2, rows3, pad3, rest[d_lo:]]
        )
        assert perm.size == I
        pos = np.empty(I, dtype=np.int64)
        pos[perm] = np.arange(I)

        # images
        img1 = np.full((I, OSH), NEG, np.float16)
        img2 = np.full((n_dirty_cap, OSH), NEG, np.float16)
        img3 = np.full((D3 * 128, OSH), NEG, np.float16)
        p_s = pos[i_s]
        r0 = rank == 0
        img1[p_s[r0], o_s[r0]] = w_s[r0].astype(np.float16)
        d_base = FIRST_DIRTY * 128
        r1 = rank == 1
        assert (p_s[r1] >= d_base).all()
        img2[p_s[r1] - d_base, o_s[r1]] = w_s[r1].astype(np.float16)
        t_base = D3_CHUNK * 128
        r2 = rank == 2
        assert (p_s[r2] >= t_base).all()
        img3[p_s[r2] - t_base, o_s[r2]] = w_s[r2].astype(np.float16)

        # packed supply stream: [xT | img1] per permuted row
        pk = np.concatenate([xT[perm], img1], axis=1)  # [I, PKW] f16
        assert pk.shape == (I, PKW)

        # side tensor: [im2 (3 chunks) | im3 | bias-as-f16-words] per partition
        im2_t = np.ascontiguousarray(
            img2.reshape(D_DIRTY, 128, OSH).transpose(1, 0, 2).reshape(
                128, D_DIRTY * OSH
            )
        )
        im3_t = img3  # [128, OSH]
        bias_t = np.ascontiguousarray(b_c.reshape(OC, 128).T)  # [128, OC] f32
        bias_w = bias_t.astype(np.float32).view(np.float16)    # [128, 2*OC]
        idn = np.full((128, 8), -1, np.int16)
        idn[:16, :] = np.arange(8)[None, :] * 16 + np.arange(16)[:, None]
        side = np.concatenate(
            [
                im2_t.view(np.int16),
                im3_t.view(np.int16),
                bias_w.view(np.int16),
                idn,
            ],
            axis=1,
        )

        in_maps.append({"pk": pk, "side": side})
    return in_maps


def kernel(x, input_selection, weights, biases):
    nc = _build()
    in_maps = _host_prep(x, input_selection, weights, biases)
    res = bass_utils.run_bass_kernel_spmd(nc, in_maps, core_ids=list(range(NCORES)))
    outT = np.concatenate(
        [np.asarray(res.results[c]["outT"]) for c in range(NCORES)], axis=0
    )  # [O, B] f16
    return np.ascontiguousarray(outT.T.astype(np.float32))  # [B, O]
